# revision 7
# baseline (speedup 1.0000x reference)
"""AttnRes pooling kernel for Trainium2 (Bass/Tile), 8-core SPMD.

Computes, for V = layer_outputs [N=12, B=4, T=2048, D=768]:
    inv_rms = rsqrt(mean(V^2, -1) + 1e-6)
    logits[n,b,t] = dot(q*w, V[n,b,t,:]) * inv_rms[n,b,t]
    alpha = softmax(logits, axis=0)   # over layer dim N
    h[b,t,d] = sum_n alpha[n,b,t] * V[n,b,t,d]

Sharding: B*T = 8192 positions split contiguously across 8 cores (1024
positions each). q*w is combined on host and replicated. Softmax is over N,
so no cross-core communication is needed.

Per-core device program (natural layout: 128 positions on partitions, D on
the free dim):
  - DVE: tensor_tensor_reduce (V * qw_broadcast -> per-position dot),
         small softmax ops, scalar_tensor_tensor (h += alpha_n * V_n).
  - ACT: activation(Square, accum_out) -> sum(V^2); Exp for softmax.
"""

from contextlib import ExitStack

import numpy as np

import concourse.bass as bass
import concourse.mybir as mybir
import concourse.tile as tile
from concourse import bass_utils

N_LAYERS = 12
B = 4
T = 2048
D = 768
N_CORES = 8
POS = B * T  # 8192
PPC = POS // N_CORES  # 1024 positions per core
P = 128  # SBUF partitions
NTILES = PPC // P  # 8 position-tiles per core
EPS = 1e-6

f32 = mybir.dt.float32


def _split_multiwaits(nc: bass.Bass) -> int:
    """Hoist all-but-one sync waits onto standalone InstEventSemaphore
    instructions inserted immediately before the over-subscribed instruction.

    This walrus build accepts only one sync-wait per TPB instruction, while
    bass_rust's Tile scheduler emits up to two on event-semaphore (HWDGE)
    waits. Inserting the extra waits as EventSemaphore instructions at the
    same program point on the same engine is semantically identical.
    """
    cnt = 0
    for f in nc.m.functions:
        for bb in f.blocks:
            insts = bb.instructions
            i = 0
            while i < len(insts):
                inst = insts[i]
                si = inst.sync_info
                if si is not None and si.on_wait is not None and len(si.on_wait) > 1:
                    waits = list(si.on_wait)
                    for j, w in enumerate(waits[:-1]):
                        ev = mybir.InstEventSemaphore(
                            name=f"{inst.name}-wsplit{j}",
                            engine=inst.engine,
                            sync_info=mybir.SyncInfo(on_wait=[w], on_update=[]),
                        )
                        insts.insert(i, ev)
                        i += 1
                        cnt += 1
                    inst.sync_info = mybir.SyncInfo(
                        on_wait=[waits[-1]], on_update=list(si.on_update or [])
                    )
                i += 1
    return cnt


def _build_bass() -> bass.Bass:
    nc = bass.Bass("TRN2")
    lo = nc.dram_tensor("lo", [N_LAYERS, PPC, D], f32, kind="ExternalInput").ap()
    qwb = nc.dram_tensor("qwb", [P, D], f32, kind="ExternalInput").ap()
    out = nc.dram_tensor("out", [PPC, D], f32, kind="ExternalOutput").ap()

    Alu = mybir.AluOpType
    Act = mybir.ActivationFunctionType

    with ExitStack() as ctx:
        tc = ctx.enter_context(tile.TileContext(nc))
        singles = ctx.enter_context(tc.tile_pool(name="singles", bufs=1))
        vpool = ctx.enter_context(tc.tile_pool(name="v", bufs=2))
        spool = ctx.enter_context(tc.tile_pool(name="small", bufs=2))
        hpool = ctx.enter_context(tc.tile_pool(name="h", bufs=2))

        qwb_t = singles.tile([P, D], f32)
        nc.sync.dma_start(out=qwb_t, in_=qwb)
        eps_t = singles.tile([P, 1], f32)
        nc.vector.memset(eps_t, EPS)

        for i in range(NTILES):
            v = vpool.tile([P, N_LAYERS, D], f32)
            for n in range(N_LAYERS):
                nc.sync.dma_start(out=v[:, n, :], in_=lo[n, i * P : (i + 1) * P, :])

            dots = spool.tile([P, N_LAYERS], f32)
            s2 = spool.tile([P, N_LAYERS], f32)
            dummy_v = spool.tile([P, 1], f32)
            dummy_a = spool.tile([P, 1], f32)
            for n in range(N_LAYERS):
                # dots[:, n] = sum_d V[:, n, d] * qw[d]   (DVE, one pass)
                nc.vector.scalar_tensor_tensor(
                    out=dummy_v.broadcast_to((P, D)),
                    in0=v[:, n, :],
                    scalar=1.0,
                    in1=qwb_t,
                    op0=Alu.mult,
                    op1=Alu.mult,
                    accum_out=dots[:, n : n + 1],
                )
                # s2[:, n] = sum_d V[:, n, d]^2   (ACT, one pass)
                nc.scalar.activation(
                    out=dummy_a.broadcast_to((P, D)),
                    in_=v[:, n, :],
                    func=Act.Square,
                    accum_out=s2[:, n : n + 1],
                )

            # rms = sqrt(s2/D + eps); logits = dots / rms
            rms = spool.tile([P, N_LAYERS], f32)
            nc.scalar.activation(
                out=rms, in_=s2, func=Act.Sqrt, scale=1.0 / D, bias=eps_t
            )
            invr = spool.tile([P, N_LAYERS], f32)
            nc.vector.reciprocal(invr, rms)
            logits = spool.tile([P, N_LAYERS], f32)
            nc.vector.tensor_mul(logits, dots, invr)

            # softmax over the N (free) dim
            negm = spool.tile([P, 1], f32)
            nc.vector.tensor_reduce(
                negm, logits, axis=mybir.AxisListType.X, op=Alu.max, negate=True
            )
            e = spool.tile([P, N_LAYERS], f32)
            se = spool.tile([P, 1], f32)
            nc.scalar.activation(
                out=e, in_=logits, func=Act.Exp, bias=negm, scale=1.0, accum_out=se
            )
            ise = spool.tile([P, 1], f32)
            nc.vector.reciprocal(ise, se)
            alpha = spool.tile([P, N_LAYERS], f32)
            nc.vector.tensor_scalar_mul(alpha, e, ise)

            # h = sum_n alpha_n * V_n  (per-partition scalar multiply-add)
            h = hpool.tile([P, D], f32)
            nc.vector.tensor_scalar_mul(h, v[:, 0, :], alpha[:, 0:1])
            for n in range(1, N_LAYERS):
                nc.vector.scalar_tensor_tensor(
                    out=h,
                    in0=v[:, n, :],
                    scalar=alpha[:, n : n + 1],
                    in1=h,
                    op0=Alu.mult,
                    op1=Alu.add,
                )
            nc.sync.dma_start(out=out[i * P : (i + 1) * P, :], in_=h)

    _split_multiwaits(nc)
    return nc


def _make_in_maps(layer_outputs, pseudo_query, key_norm_weight):
    V = np.ascontiguousarray(np.asarray(layer_outputs, dtype=np.float32)).reshape(
        N_LAYERS, POS, D
    )
    qw = np.asarray(pseudo_query, dtype=np.float32) * np.asarray(
        key_norm_weight, dtype=np.float32
    )
    qwb = np.ascontiguousarray(np.broadcast_to(qw[None, :], (P, D))).astype(np.float32)
    in_maps = []
    for c in range(N_CORES):
        shard = np.ascontiguousarray(V[:, c * PPC : (c + 1) * PPC, :])
        in_maps.append({"lo": shard, "qwb": qwb})
    return in_maps


def kernel(layer_outputs, pseudo_query, key_norm_weight):
    nc = _build_bass()
    in_maps = _make_in_maps(layer_outputs, pseudo_query, key_norm_weight)
    res = bass_utils.run_bass_kernel_spmd(nc, in_maps, core_ids=list(range(N_CORES)))
    outs = [r["out"] for r in res.results]
    return np.concatenate(outs, axis=0).reshape(B, T, D).astype(np.float32)


# revision 9
# speedup vs baseline: 109507.1147x; 109507.1147x over previous
"""AttnRes pooling kernel for Trainium2 (Bass/Tile), 8-core SPMD.

Computes, for V = layer_outputs [N=12, B=4, T=2048, D=768]:
    inv_rms = rsqrt(mean(V^2, -1) + 1e-6)
    logits[n,b,t] = dot(q*w, V[n,b,t,:]) * inv_rms[n,b,t]
    alpha = softmax(logits, axis=0)   # over layer dim N
    h[b,t,d] = sum_n alpha[n,b,t] * V[n,b,t,d]

Sharding: B*T = 8192 positions split contiguously across 8 cores (1024
positions each). q*w is combined on host and replicated. Softmax is over N,
so no cross-core communication is needed.

Per-core device program (natural layout: 128 positions on partitions, D on
the free dim):
  - DVE: tensor_tensor_reduce (V * qw_broadcast -> per-position dot),
         small softmax ops, scalar_tensor_tensor (h += alpha_n * V_n).
  - ACT: activation(Square, accum_out) -> sum(V^2); Exp for softmax.
"""

from contextlib import ExitStack

import numpy as np

import concourse.bass as bass
import concourse.mybir as mybir
import concourse.tile as tile
from concourse import bass_utils

N_LAYERS = 12
B = 4
T = 2048
D = 768
N_CORES = 8
POS = B * T  # 8192
PPC = POS // N_CORES  # 1024 positions per core
P = 128  # SBUF partitions
NTILES = PPC // P  # 8 position-tiles per core
EPS = 1e-6

f32 = mybir.dt.float32


def _split_multiwaits(nc: bass.Bass) -> int:
    """Hoist all-but-one sync waits onto standalone InstEventSemaphore
    instructions inserted immediately before the over-subscribed instruction.

    This walrus build accepts only one sync-wait per TPB instruction, while
    bass_rust's Tile scheduler emits up to two on event-semaphore (HWDGE)
    waits. Inserting the extra waits as EventSemaphore instructions at the
    same program point on the same engine is semantically identical.
    """
    cnt = 0
    for f in nc.m.functions:
        for bb in f.blocks:
            insts = bb.instructions
            i = 0
            while i < len(insts):
                inst = insts[i]
                si = inst.sync_info
                if si is not None and si.on_wait is not None and len(si.on_wait) > 1:
                    waits = list(si.on_wait)
                    for j, w in enumerate(waits[:-1]):
                        ev = mybir.InstEventSemaphore(
                            name=f"{inst.name}-wsplit{j}",
                            engine=inst.engine,
                            sync_info=mybir.SyncInfo(on_wait=[w], on_update=[]),
                        )
                        insts.insert(i, ev)
                        i += 1
                        cnt += 1
                    inst.sync_info = mybir.SyncInfo(
                        on_wait=[waits[-1]], on_update=list(si.on_update or [])
                    )
                i += 1
    return cnt


def _build_bass(reps: int = 1) -> bass.Bass:
    nc = bass.Bass("TRN2")
    lo = nc.dram_tensor("lo", [N_LAYERS, PPC, D], f32, kind="ExternalInput").ap()
    qwb = nc.dram_tensor("qwb", [P, D], f32, kind="ExternalInput").ap()
    out = nc.dram_tensor("out", [PPC, D], f32, kind="ExternalOutput").ap()

    Alu = mybir.AluOpType
    Act = mybir.ActivationFunctionType

    with ExitStack() as ctx:
        tc = ctx.enter_context(tile.TileContext(nc))
        singles = ctx.enter_context(tc.tile_pool(name="singles", bufs=1))
        vpool = ctx.enter_context(tc.tile_pool(name="v", bufs=2))
        spool = ctx.enter_context(tc.tile_pool(name="small", bufs=2))
        hpool = ctx.enter_context(tc.tile_pool(name="h", bufs=2))

        qwb_t = singles.tile([P, D], f32)
        nc.sync.dma_start(out=qwb_t, in_=qwb)
        eps_t = singles.tile([P, 1], f32)
        nc.vector.memset(eps_t, EPS)

        for i in [t for _ in range(reps) for t in range(NTILES)]:
            v = vpool.tile([P, N_LAYERS, D], f32)
            for n in range(N_LAYERS):
                nc.sync.dma_start(out=v[:, n, :], in_=lo[n, i * P : (i + 1) * P, :])

            dots = spool.tile([P, N_LAYERS], f32)
            s2 = spool.tile([P, N_LAYERS], f32)
            dummy_v = spool.tile([P, 1], f32)
            dummy_a = spool.tile([P, 1], f32)
            for n in range(N_LAYERS):
                # dots[:, n] = sum_d V[:, n, d] * qw[d]   (DVE, one pass)
                nc.vector.scalar_tensor_tensor(
                    out=dummy_v.broadcast_to((P, D)),
                    in0=v[:, n, :],
                    scalar=1.0,
                    in1=qwb_t,
                    op0=Alu.mult,
                    op1=Alu.mult,
                    accum_out=dots[:, n : n + 1],
                )
                # s2[:, n] = sum_d V[:, n, d]^2   (ACT, one pass)
                nc.scalar.activation(
                    out=dummy_a.broadcast_to((P, D)),
                    in_=v[:, n, :],
                    func=Act.Square,
                    accum_out=s2[:, n : n + 1],
                )

            # rms = sqrt(s2/D + eps); logits = dots / rms
            rms = spool.tile([P, N_LAYERS], f32)
            nc.scalar.activation(
                out=rms, in_=s2, func=Act.Sqrt, scale=1.0 / D, bias=eps_t
            )
            invr = spool.tile([P, N_LAYERS], f32)
            nc.vector.reciprocal(invr, rms)
            logits = spool.tile([P, N_LAYERS], f32)
            nc.vector.tensor_mul(logits, dots, invr)

            # softmax over the N (free) dim
            negm = spool.tile([P, 1], f32)
            nc.vector.tensor_reduce(
                negm, logits, axis=mybir.AxisListType.X, op=Alu.max, negate=True
            )
            e = spool.tile([P, N_LAYERS], f32)
            se = spool.tile([P, 1], f32)
            nc.scalar.activation(
                out=e, in_=logits, func=Act.Exp, bias=negm, scale=1.0, accum_out=se
            )
            ise = spool.tile([P, 1], f32)
            nc.vector.reciprocal(ise, se)
            alpha = spool.tile([P, N_LAYERS], f32)
            nc.vector.tensor_scalar_mul(alpha, e, ise)

            # h = sum_n alpha_n * V_n  (per-partition scalar multiply-add)
            h = hpool.tile([P, D], f32)
            nc.vector.tensor_scalar_mul(h, v[:, 0, :], alpha[:, 0:1])
            for n in range(1, N_LAYERS):
                nc.vector.scalar_tensor_tensor(
                    out=h,
                    in0=v[:, n, :],
                    scalar=alpha[:, n : n + 1],
                    in1=h,
                    op0=Alu.mult,
                    op1=Alu.add,
                )
            nc.sync.dma_start(out=out[i * P : (i + 1) * P, :], in_=h)

    _split_multiwaits(nc)
    return nc


def _make_in_maps(layer_outputs, pseudo_query, key_norm_weight):
    V = np.ascontiguousarray(np.asarray(layer_outputs, dtype=np.float32)).reshape(
        N_LAYERS, POS, D
    )
    qw = np.asarray(pseudo_query, dtype=np.float32) * np.asarray(
        key_norm_weight, dtype=np.float32
    )
    qwb = np.ascontiguousarray(np.broadcast_to(qw[None, :], (P, D))).astype(np.float32)
    in_maps = []
    for c in range(N_CORES):
        shard = np.ascontiguousarray(V[:, c * PPC : (c + 1) * PPC, :])
        in_maps.append({"lo": shard, "qwb": qwb})
    return in_maps


def kernel(layer_outputs, pseudo_query, key_norm_weight):
    nc = _build_bass()
    in_maps = _make_in_maps(layer_outputs, pseudo_query, key_norm_weight)
    res = bass_utils.run_bass_kernel_spmd(nc, in_maps, core_ids=list(range(N_CORES)))
    outs = [r["out"] for r in res.results]
    return np.concatenate(outs, axis=0).reshape(B, T, D).astype(np.float32)
